# revision 1
# baseline (speedup 1.0000x reference)
"""Cross-modal attention Trainium2 kernel (fp8 DoubleRow rewrite).

Sharding: 8 cores, one per (direction, batch, query-half):
  core = dir*4 + b*2 + qh
  dir 0: out1 rows (q from x1, k/v from x2); dir 1: out2 (q from x2, k/v from x1)

All GEMMs run in fp8e4m3 DoubleRow mode (2 contraction k-tiles per
instruction, 0.5 cycles per output row):
  - q/k/v projections contract over c in pairs of 128-chunks (host ships x
    and W with c-chunk pairs interleaved on the free axis).
  - scores per head contract over D=64 using a zero second plane (qt carries
    a DMA-zeroed plane; the zero side makes the second k-tile a no-op).
  - attn@v is computed transposed (out [q, d]) with exp'd scores stationary
    and v moving; v is split hi+lo fp8 on device (residual) so the value
    path keeps ~bf16 accuracy. A 1/64 ones-column on v_hi yields the softmax
    denominator scaled so fp8 att values avoid e4m3 subnormals.
  - out-proj: att (fp8, x64 scale) stationary after PE transposes, wo hi+lo
    fp8 moving; final evac folds 1/(64*16) and the output bias.
Weights are prescaled x16 on host (e4m3 subnormal avoidance); 1/16 folds
into the PSUM evacuations. The 1/sqrt(D) score scale folds into the exp.

exp is split across engines: ACT runs native Exp (scale=1/8) straight to
fp8e4; DVE and Pool run a quantized-Schraudolph bit trick:
  u8 = round(score * log2(e)/8 * 8 + 55.54)  viewed as fp8e4 bits
which lands within ~3.1% rms of exp (vs 2.65% for exact exp->e4m3).

Timing-relevant structure: scores for head h stream while attnv(h-2)
accumulates in PSUM and projections drain through leftover PE slots; the
three exp engines are the per-head pacing item.
"""

import sys

sys.path.insert(0, "/opt/trn_rl_repo")

import numpy as np
import ml_dtypes

EMBED = 512
H = 8
D = 64
B = 2
L = 2048
LQ = 1024

SW = 16.0  # weight prescale (host)
SA = 64.0  # att prescale via 1/SA ones-column
C1 = float(np.log2(np.e) * 8.0 / 8.0)  # schraudolph mult (incl 1/sqrt(D))
C2 = 56.0 - 0.46  # schraudolph magic bias

f8np = ml_dtypes.float8_e4m3
bfnp = ml_dtypes.bfloat16

_CACHE = {}

# exp engine per j (A=ACT native, D=DVE schraudolph, P=Pool schraudolph)
EXP_PAT = "ADADADAADADADAADADADADDADADADAAD"  # per (j,ih): A17 D15
# norm engine per qc
NORM_PAT = "DPDPDPDP"
# transpose-evac engine rotation
TEV_PAT = "DA"
# y evac engine rotation
YEV_PAT = "DA"


def _build_nc(abl=0):
    import concourse.bacc as bacc
    import concourse.mybir as mybir
    import concourse.tile as tile

    BF = mybir.dt.bfloat16
    F32 = mybir.dt.float32
    F8 = mybir.dt.float8e4
    U8 = mybir.dt.uint8
    EXP = mybir.ActivationFunctionType.Exp
    IDENT = mybir.ActivationFunctionType.Identity
    DR = mybir.MatmulPerfMode.DoubleRow
    AL = mybir.AluOpType

    nc = bacc.Bacc("TRN2", target_bir_lowering=False)

    xq_d = nc.dram_tensor("xq", [128, 2, 2, LQ], F8, kind="ExternalInput")
    xk_d = nc.dram_tensor("xk", [2, 2, 128, 2, L], F8, kind="ExternalInput")
    w1_d = nc.dram_tensor("w1", [128, 2, 2, 2, 512], F8, kind="ExternalInput")
    w2_d = nc.dram_tensor("w2", [128, 2, 2, 2, 512], F8, kind="ExternalInput")
    w3_d = nc.dram_tensor("w3", [128, 2, 2, 2, 512], BF, kind="ExternalInput")
    bqk_d = nc.dram_tensor("bqk", [128, 8], F32, kind="ExternalInput")
    bo_d = nc.dram_tensor("bo", [1, 512], F32, kind="ExternalInput")
    bo2_d = nc.dram_tensor("bo2", [2, 512], BF, kind="ExternalInput")
    idn_d = nc.dram_tensor("idn", [128, 128], BF, kind="ExternalInput")
    zro_d = nc.dram_tensor("zro", [128, 4 * LQ], F8, kind="ExternalInput")
    y_d = nc.dram_tensor("y", [8, 128, 512], F32, kind="ExternalOutput")

    with tile.TileContext(nc) as tc:
        with tc.tile_pool(name="persist", bufs=1) as pp:
            xq = pp.tile([128, 2, 2, LQ], F8, name="xq")
            xk = pp.tile([128, 2, 2, 2, L], F8, name="xk")
            w1 = pp.tile([128, 2, 2, 2, 512], F8, name="w1")
            w2 = pp.tile([128, 2, 2, 2, 512], F8, name="w2")
            w3 = pp.tile([128, 2, 2, 2, 512], BF, name="w3")
            bqk = pp.tile([128, 8], F32, name="bqk")
            bob = pp.tile([128, 512], F32, name="bob")
            bo2 = pp.tile([2, 512], BF, name="bo2")
            ones_r = pp.tile([2, 128], BF, name="ones_r")
            idn = pp.tile([128, 128], BF, name="idn")
            ones = pp.tile([128, 2, 2], F8, name="ones")
            qt = pp.tile([128, 4, 2, LQ], F8, name="qt")
            kt = pp.tile([128, 4, L], F8, name="kt")
            vh = pp.tile([128, 8, 2, H, D], F8, name="vh")
            vl = pp.tile([128, 8, 2, H, D], F8, name="vl")
            att = [pp.tile([128, H, D], BF, name=f"att{qc}") for qc in range(8)]
            atT = [pp.tile([128, 2, LQ], BF, name=f"atT{c}") for c in range(2)]
            wup = pp.tile([128, 512], BF, name="wup")

            nc.sync.dma_start(out=bqk, in_=bqk_d[:])
            nc.sync.dma_start(out=xq, in_=xq_d[:])
            nc.scalar.dma_start(out=w1, in_=w1_d[:])
            for hl in range(2):
                for cp in range(2):
                    e = nc.scalar if (hl + cp) % 2 else nc.sync
                    e.dma_start(out=xk[:, hl, cp], in_=xk_d[hl, cp])
            nc.sync.dma_start(out=w2, in_=w2_d[:])
            nc.gpsimd.memset(qt[:, :, 1, :], 0.0)
            nc.sync.dma_start(out=w3, in_=w3_d[:])
            nc.sync.dma_start(out=idn, in_=idn_d[:])
            nc.sync.dma_start(out=bo2, in_=bo2_d[:])
            nc.gpsimd.memset(ones_r, 1.0)
            nc.gpsimd.memset(ones, 1.0 / SA)

            with (
                tc.tile_pool(name="scps", bufs=6, space="PSUM") as scps,
                tc.tile_pool(name="avp", bufs=2, space="PSUM") as avp,
                tc.tile_pool(name="exp", bufs=16) as expool,
                tc.tile_pool(name="nrm", bufs=6) as nrm,
                tc.tile_pool(name="yst", bufs=4) as yst,
            ):
                dm = nrm.tile([1, 2], F32, name="dm")
                nc.vector.memset(dm, 0.0)
                dm2 = nrm.tile([1, 2], F32, name="dm2")
                nc.scalar.activation(dm2, dm, EXP)
                nc.vector.memset(wup, 0.0)
                wps = scps.tile([128, 512], F32, name="sc")
                for i in range(8):
                    nc.tensor.matmul(
                        wps, wup[:, 0:128], wup, start=(i == 0), stop=(i == 7)
                    )

                ex_t = {}
                av_t = {}
                den_t = {}
                rc_t = {}

                def qk_proj(f):
                    for ih in range(2):
                        ps = scps.tile([128, 512], F32, name="sc")
                        for cp in range(2):
                            nc.tensor.matmul(
                                ps,
                                w1[:, 0, cp, :, f * 128 : (f + 1) * 128],
                                xq[:, cp, :, ih * 512 : (ih + 1) * 512],
                                start=(cp == 0),
                                stop=(cp == 1),
                                perf_mode=DR,
                            )
                        nc.scalar.activation(
                            qt[:, f, 0, ih * 512 : (ih + 1) * 512],
                            ps,
                            IDENT,
                            bias=bqk[:, f : f + 1],
                            scale=1.0 / SW,
                        )
                    for th in range(4):
                        ps = scps.tile([128, 512], F32, name="sc")
                        for cp in range(2):
                            nc.tensor.matmul(
                                ps,
                                w1[:, 1, cp, :, f * 128 : (f + 1) * 128],
                                xk[:, 0, cp, :, th * 512 : (th + 1) * 512],
                                start=(cp == 0),
                                stop=(cp == 1),
                                perf_mode=DR,
                            )
                        # k bias is softmax-invariant (constant per query): skip
                        if th % 2 == 0:
                            nc.vector.tensor_scalar(
                                kt[:, f, th * 512 : (th + 1) * 512],
                                ps, 1.0 / SW, 0.0, AL.mult, AL.add,
                            )
                        else:
                            nc.scalar.mul(
                                kt[:, f, th * 512 : (th + 1) * 512], ps, 1.0 / SW
                            )

                def v_proj(t):
                    ps = scps.tile([128, 512], F32, name="sc")
                    first = True
                    for xi, wi in ((0, 0), (0, 1), (1, 0)):
                        for cp in range(2):
                            nc.tensor.matmul(
                                ps,
                                xk[:, xi, cp, :, t * 128 : (t + 1) * 128],
                                w2[:, wi, cp],
                                start=first,
                                stop=(xi == 1 and cp == 1),
                                perf_mode=DR,
                            )
                            first = False
                    p, pl = t // 2, t % 2
                    hi = vh[:, p, pl, :, :]
                    nc.scalar.mul(hi, ps.rearrange("p (h d) -> p h d", h=H), 1.0 / SW)
                    nc.vector.scalar_tensor_tensor(
                        vl[:, p, pl, :, :],
                        ps.rearrange("p (h d) -> p h d", h=H),
                        1.0 / SW, hi, AL.mult, AL.subtract,
                    )

                def score(h, j):
                    f, base = h // 2, 64 * (h % 2)
                    jp, pl = j // 2, j % 2
                    if pl == 0:
                        ex_t[(h, jp)] = expool.tile([128, 2, LQ], F8, name="ex")
                    ex = ex_t[(h, jp)]
                    lhs = kt[base : base + 64, f, j * 128 : (j + 1) * 128]
                    lhs = lhs.unsqueeze(1).broadcast_to((64, 2, 128))
                    for ih in range(2):
                        ps = scps.tile([128, 512], F32, name="sc")
                        nc.tensor.matmul(
                            ps,
                            lhs,
                            qt[base : base + 64, f, :, ih * 512 : (ih + 1) * 512],
                            start=True,
                            stop=True,
                            perf_mode=DR,
                        )
                        if abl >= 3:
                            continue
                        sl = slice(ih * 512, (ih + 1) * 512)
                        eng = EXP_PAT[(j * 2 + ih + h * 2) % 32]
                        if eng == "A":
                            nc.scalar.activation(
                                ex[:, pl, sl], ps, EXP, scale=1.0 / 8.0
                            )
                        else:
                            nc.vector.tensor_scalar(
                                ex.bitcast(U8)[:, pl, sl], ps, C1, C2,
                                AL.mult, AL.add,
                            )

                def attnv(h, qc):
                    if qc == 0:
                        av_t[h] = avp.tile([128, 8, D], F32, name="av")
                        den_t[h] = scps.tile([128, 8, 2], F32, name="sc")
                    av = av_t[h][:, qc, :]
                    dn = den_t[h][:, qc, :]
                    for jp in range(8):
                        ex = ex_t[(h, jp)][:, :, qc * 128 : (qc + 1) * 128]
                        for gi, vv in enumerate((vh, vl)):
                            nc.tensor.matmul(
                                av,
                                ex,
                                vv[:, jp, :, h, :],
                                start=(jp == 0 and gi == 0),
                                stop=(jp == 7 and gi == 1),
                                perf_mode=DR,
                            )
                        nc.tensor.matmul(
                            dn, ex, ones,
                            start=(jp == 0), stop=(jp == 7), perf_mode=DR,
                        )
                    if qc == 7:
                        for jp in range(8):
                            del ex_t[(h, jp)]

                def norm(h):
                    rc = nrm.tile([128, 8], F32, name="rc")
                    rc_t[h] = rc
                    nc.vector.reciprocal(rc, den_t[h][:, :, 0:1])
                    del den_t[h]
                    avsb = nrm.tile([128, 8, D], BF, name="avsb")
                    nc.vector.tensor_copy(avsb, av_t[h])
                    for qc in range(8):
                        nc.gpsimd.tensor_scalar(
                            att[qc][:, h, :], avsb[:, qc, :],
                            rc[:, qc : qc + 1], 1.0, AL.mult, AL.mult,
                        )
                    del av_t[h]

                def transp1(cc, qc):
                    tp = scps.tile([128, 128], BF, name="sc")
                    nc.tensor.transpose(
                        tp, att[qc][:, 2 * cc : 2 * cc + 2, :], idn
                    )
                    e = {"D": nc.vector, "A": nc.scalar}[
                        TEV_PAT[(cc * 8 + qc) % 2]
                    ]
                    dst = atT[cc // 2][:, cc % 2, qc * 128 : (qc + 1) * 128]
                    if e is nc.scalar:
                        nc.scalar.copy(dst, tp)
                    else:
                        e.tensor_copy(dst, tp)

                def transp(cc):
                    for qc in range(8):
                        transp1(cc, qc)

                # ---- schedule --------------------------------------------
                qk_proj(0)
                qk_proj(1)
                vq = list(range(16))

                def drain_v(n):
                    for _ in range(min(n, len(vq))):
                        v_proj(vq.pop(0))

                for h in range(8):
                    for j in range(16):
                        score(h, j)
                        if h == 0 and j in (3, 9):
                            qk_proj(2 + (j > 4))
                        if h < 1:
                            drain_v(2)
                        if abl >= 2:
                            continue
                        if h >= 2 and j == 2 and abl < 1:
                            norm(h - 2)
                        if h >= 1 and j == 7:
                            for qc in range(4):
                                attnv(h - 1, qc)
                        if h >= 1 and j == 12:
                            for qc in range(4, 8):
                                attnv(h - 1, qc)
                # tail: cc 0-2 transposes ride along attnv(7); each
                # out-proj starts right after its own cc=3 transpose.
                if abl < 2:
                    if abl < 1:
                        norm(6)
                    for qc in range(8):
                        attnv(7, qc)
                        if abl < 1:
                            for cc in range(3):
                                transp1(cc, qc)
                if abl < 1:
                    norm(7)

                    for t in range(8):
                        transp1(3, t)
                        ps = scps.tile([128, 512], F32, name="sc")
                        i = 0
                        for cp in range(2):
                            for pl in range(2):
                                nc.tensor.matmul(
                                    ps,
                                    atT[cp][:, pl, t * 128 : (t + 1) * 128],
                                    w3[:, 0, cp, pl, :],
                                    start=(i == 0),
                                    stop=False,
                                )
                                i += 1
                        # bias as a 2-row (hi/lo bf16) constant contraction
                        nc.tensor.matmul(
                            ps, ones_r, bo2, start=False, stop=True
                        )
                        ysb = yst.tile([128, 512], F32, name="ysb")
                        e = {"D": nc.vector, "A": nc.scalar}[YEV_PAT[t % 2]]
                        if e is nc.scalar:
                            nc.scalar.mul(ysb, ps, 1.0 / SA)
                        else:
                            nc.vector.tensor_scalar(
                                ysb, ps, 1.0 / SA, 0.0, AL.mult, AL.add
                            )
                        (nc.sync if t % 2 == 0 else nc.scalar).dma_start(
                            out=y_d[t], in_=ysb
                        )

    nc.finalize()
    return nc


def _bo2(b):
    hi = b.astype(bfnp)
    lo = (b - hi.astype(np.float32)).astype(bfnp)
    return np.stack([hi, lo], axis=0).reshape(2, 512)


def _pairplane(a):
    # [512, N] -> [2, 128, 2, N] with c-chunk pairs interleaved on planes
    n = a.shape[1]
    return np.ascontiguousarray(
        a.reshape(2, 2, 128, n).transpose(0, 2, 1, 3)
    )


def _prep_weights(qkv_w, qkv_b, out_w, out_b):
    w = qkv_w.reshape(H, 3, D, EMBED)
    b3 = qkv_b.reshape(H, 3, D)
    wq = w[:, 0].reshape(EMBED, EMBED)
    wk = w[:, 1].reshape(EMBED, EMBED)
    wv = w[:, 2].reshape(EMBED, EMBED)
    bq = b3[:, 0].reshape(EMBED)
    bv = b3[:, 2].reshape(EMBED)

    def hilo(m):
        hi = (m * SW).astype(f8np)
        lo = (m * SW - hi.astype(np.float32)).astype(f8np)
        return hi, lo

    wq8 = (wq.T.astype(np.float32) * SW).astype(f8np)
    wk8 = (wk.T.astype(np.float32) * SW).astype(f8np)
    wvh, wvl = hilo(wv.T.astype(np.float32))
    wo16 = out_w.T.astype(np.float32).astype(bfnp)

    def pack2(a, b):
        # two [512, 512] f8 -> [128, 2(which), 2(cp), 2(plane), 512]
        s = np.stack([_pairplane(a), _pairplane(b)], axis=0)  # [w, cp, 128, pl, f]
        return np.ascontiguousarray(s.transpose(2, 0, 1, 3, 4))

    bqk = np.zeros((128, 8), np.float32)
    for f in range(4):
        bqk[:, f] = bq[f * 128 : (f + 1) * 128]
    return {
        "w1": pack2(wq8, wk8),
        "w2": pack2(wvh, wvl),
        "w3": pack2(wo16, wo16),
        "bqk": bqk,
        "bo": (out_b + out_w @ bv).astype(np.float32).reshape(1, 512),
        "bo2": _bo2((out_b + out_w @ bv).astype(np.float32) * SA),
        "idn": np.eye(128, dtype=np.float32).astype(bfnp),
        "zro": np.zeros((128, 4 * LQ), f8np),
    }


def _make_in_maps(x1, x2, shared):
    xT = {}
    for mod, x in ((0, x1), (1, x2)):
        for b in range(B):
            t = np.ascontiguousarray(x[b].T).astype(np.float32)
            hi = t.astype(f8np)
            lo = (t - hi.astype(np.float32)).astype(f8np)
            xT[(mod, b)] = (hi, lo)
    in_maps = []
    for core in range(8):
        d, b, qh = core // 4, (core // 2) % 2, core % 2
        hi_q = xT[(d, b)][0][:, qh * LQ : (qh + 1) * LQ]
        hi_kv, lo_kv = xT[(1 - d, b)]
        m = dict(shared)
        m["xq"] = np.ascontiguousarray(
            _pairplane(np.ascontiguousarray(hi_q)).transpose(1, 0, 2, 3)
        )
        m["xk"] = np.ascontiguousarray(
            np.stack([_pairplane(hi_kv), _pairplane(lo_kv)], axis=0)
        )
        in_maps.append(m)
    return in_maps


def kernel(x1, x2, qkv_w, qkv_b, out_w, out_b):
    from concourse.bass_utils import run_bass_kernel_spmd

    x1 = np.asarray(x1, dtype=np.float32)
    x2 = np.asarray(x2, dtype=np.float32)
    shared = _prep_weights(
        np.asarray(qkv_w, np.float32),
        np.asarray(qkv_b, np.float32),
        np.asarray(out_w, np.float32),
        np.asarray(out_b, np.float32),
    )
    in_maps = _make_in_maps(x1, x2, shared)

    if "nc" not in _CACHE:
        _CACHE["nc"] = _build_nc()
    try:
        res = run_bass_kernel_spmd(_CACHE["nc"], in_maps, core_ids=list(range(8)))
    except Exception:
        res = run_bass_kernel_spmd(_CACHE["nc"], in_maps, core_ids=list(range(8)))

    out1 = np.empty((B, L, EMBED), np.float32)
    out2 = np.empty((B, L, EMBED), np.float32)
    outs = {0: out1, 1: out2}
    for core in range(8):
        d, b, qh = core // 4, (core // 2) % 2, core % 2
        yc = res.results[core]["y"].reshape(LQ, EMBED)
        outs[d][b, qh * LQ : (qh + 1) * LQ, :] = yc
    return out1, out2

